# revision 6
# baseline (speedup 1.0000x reference)
"""Trainium2 Bass kernel for 2D single-level DWT (coif1, symmetric padding).

Input  x: (4, 64, 512, 512) fp32
Output  : (4, 256, 258, 258) fp32  -- per input channel: [cA, cH, cV, cD]

Math: with R_f the banded 258x512 operator of the 1D DWT along an axis
(6-tap filter, stride 2, symmetric boundary folds), the four outputs are
    cA = R_lo X R_lo^T,  cH = R_hi X R_lo^T,
    cV = R_lo X R_hi^T,  cD = R_hi X R_hi^T.

v5 design (per-image pipeline, contiguous 1-bank PSUM chains):
  pass 1 (contract rows r):   Yt_f[c, kh] = sum_r X[r, c] R_f[kh, r]
     stationary lhsT = X chunk [r:128, c:128]; moving rhs = R^T slice with
     the lo/hi filter pair interleaved along the stream dim (col 2*kh+f).
     R is banded: r-chunk q only reaches khf cols [128q, 128q+132).
  pass 2 (contract cols c):   O[kh, kwg] = sum_c Yt_f[c, kh] R_g[kw, c]
     stationary lhsT = stride-2 slice of Yt (kh chunk for filter f);
     moving rhs = the SAME banded weight tensor.
  Each chain writes khf/kwg cols [0,512) into ONE psum bank and the
  4-col tail [512,516) into a slot of a shared tail bank, so the main
  drain is a single contiguous [128,512] fp32->fp16 copy (strided drains
  cost ~2x on ACT/DVE).  Tail slots are drained once per image with one
  small strided copy.  Drains are split across scalar+vector by time.
  The PE runs pass1(i+1) before pass2(i) so drains trail a full image
  behind the producer.  DMA: per-image transfers (input 2 halves on the
  sync queue, output on gpsimd), input prefetch depth 5.
  A post-legalize pass drops LDWEIGHTS whose weights AP equals the
  previous load on the final PE stream (the PE keeps its stationary
  across matmuls), saving ~105ns of queue occupancy each.
"""

import os
import sys

for _p in ("/opt/trn_rl_repo", "/opt/pypackages"):
    if _p not in sys.path:
        sys.path.append(_p)

os.environ.setdefault("JAX_COMPILATION_CACHE_DIR", "/tmp/jax_comp_cache")
os.environ.setdefault("JAX_PERSISTENT_CACHE_MIN_COMPILE_TIME_SECS", "10")

import numpy as np

import concourse.bass as bass
import concourse.bacc as bacc
import concourse.mybir as mybir
import concourse.tile as _tile_mod
from concourse.bass_utils import run_bass_kernel_spmd
from concourse.tile import TileContext

N_CORES = 8
H = W = 512
OUT = 258  # (512 + 6 - 1) // 2
IMGS = 32  # images per core (4*64/8)
F16 = mybir.dt.float16
F32 = mybir.dt.float32

# pywt coif1 decomposition filters
DEC_LO = np.array([-0.01565572813546454, -0.0727326195128539, 0.38486484686420286,
                   0.8525720202122554, 0.3378976624578092, -0.0727326195128539])
DEC_HI = np.array([0.0727326195128539, 0.3378976624578092, -0.8525720202122554,
                   0.38486484686420286, 0.0727326195128539, -0.01565572813546454])
FLEN = 6
PAD = 4
LO_F = DEC_LO[::-1]
HI_F = DEC_HI[::-1]

# kh/kw window that r/c-chunk q contributes to (from the band structure)
WINS = [(0, 66), (64, 130), (128, 194), (192, 258)]


def _dedup_ldweights(ordered, nc):
    """Drop InstLdweights whose weights AP is identical to the previous
    PE weight load in the final post-schedule order (only matmuls in
    between).  The PE array keeps its stationary operand across matmuls,
    so the reload is a no-op; removal is done only when the candidate's
    dep edges are covered by the live load and nothing depends on it."""
    PE = mybir.EngineType.PE
    for bb, insts in ordered.items():
        cur_sig = None
        cur_deps = None
        keep = []
        for inst in insts:
            if getattr(inst, "engine", None) != PE:
                keep.append(inst)
                continue
            tn = type(inst).__name__
            if tn == "InstLdweights":
                sig = str(inst.ins[0])
                deps = (frozenset(inst.sync_dependency_names()),
                        frozenset(inst.nosync_dependency_names()))
                if (cur_sig is not None and sig == cur_sig
                        and deps[0] <= cur_deps[0] and deps[1] <= cur_deps[1]
                        and not inst.descendants):
                    continue
                cur_sig, cur_deps = sig, deps
                keep.append(inst)
            elif tn == "InstMatmult":
                if inst.is_transpose:
                    cur_sig = None
                keep.append(inst)
            else:
                keep.append(inst)
        ordered[bb] = keep
    return ordered


_orig_tile_legalize = _tile_mod.tile_legalize


def _legalize_with_dedup(ordered, nc):
    return _dedup_ldweights(_orig_tile_legalize(ordered, nc), nc)


_tile_mod.tile_legalize = _legalize_with_dedup


def _build_R(filt: np.ndarray, n: int = W) -> np.ndarray:
    """Banded [258, 512] operator: out[k] = sum_j filt[j] * x[sym(2k + j - PAD)]."""
    out_len = (n + FLEN - 1) // 2

    def sym(i: int) -> int:
        while i < 0 or i >= n:
            if i < 0:
                i = -i - 1
            if i >= n:
                i = 2 * n - 1 - i
        return i

    R = np.zeros((out_len, n), dtype=np.float64)
    for k in range(out_len):
        for j in range(FLEN):
            R[k, sym(2 * k + j - PAD)] += filt[j]
    return R


def _build_weights() -> np.ndarray:
    """Interleaved: w[p, q*516 + 2k + f] = R_f[k, 128q + p], [128, 4*516] fp16."""
    Rs = [_build_R(LO_F), _build_R(HI_F)]
    w = np.zeros((128, 4 * 2 * OUT), dtype=np.float32)
    for q in range(4):
        blk = np.zeros((128, OUT, 2), dtype=np.float32)
        for f in range(2):
            blk[:, :, f] = Rs[f][:, 128 * q:128 * (q + 1)].T
        w[:, q * 2 * OUT:(q + 1) * 2 * OUT] = blk.reshape(128, 2 * OUT)
    return w.astype(np.float16)


_WEIGHTS = _build_weights()
_MODULE = None

# chain segments: (q, psum col lo, psum col hi) over the khf/kwg index
# [0,516); q's band is [128q, 128q+132), clipped at the 512 bank edge.
# The [512,516) remainder goes to a tail-bank slot.
SEGS_MAIN = [(0, 0, 132), (1, 128, 260), (2, 256, 388), (3, 384, 512)]
SEG_TAIL = (3, 512, 516)


def _wslice(Wr, q, lo, hi):
    base = q * 516 + 128 * q
    return Wr[:, base + (lo - 128 * q):base + (hi - 128 * q)]


def _build_module() -> bass.Bass:
    nc = bacc.Bacc("TRN2", target_bir_lowering=False, debug=False)
    x_in = nc.declare_dram_parameter("x", [IMGS, 128, 4 * W], F16, isOutput=False)
    w_in = nc.declare_dram_parameter("w", [128, 4 * 516], F16, isOutput=False)
    # y[i, p, (2*khc+f)*516 + 2*kw + g] = O_{f+2g}[128*khc + p, kw]
    y_out = nc.declare_dram_parameter("y", [IMGS, 128, 4 * 516], F16,
                                      isOutput=True)
    # yr[2*j + f, i*516 + 2*kw + g] = O_{f+2g}[256 + j, kw]
    yr_out = nc.declare_dram_parameter("yr", [4, IMGS * 516], F16,
                                       isOutput=True)

    PREF = 5  # input prefetch depth (images)

    with TileContext(nc) as tc:
        with (
            tc.tile_pool(name="wpool", bufs=1) as wpool,
            tc.tile_pool(name="xpool", bufs=PREF) as xpool,
            tc.tile_pool(name="ypool", bufs=3) as ypool,
            tc.tile_pool(name="spool", bufs=3) as spool,
            tc.tile_pool(name="rpool", bufs=1) as rpool,
            tc.tile_pool(name="p1", bufs=3, space="PSUM") as p1pool,
            tc.tile_pool(name="p2", bufs=3, space="PSUM") as p2pool,
            tc.tile_pool(name="pr", bufs=1, space="PSUM") as prpool,
            tc.tile_pool(name="pt", bufs=1, space="PSUM") as ptpool,
        ):
            Wt = wpool.tile([128, 4 * 516], F16)
            Crem = rpool.tile([4, IMGS * 516], F16)
            # shared tail bank: slots 0-3 pass1(cc), 4-7 pass2(blk), 8 rem
            Tb = ptpool.tile([128, 40], F32)

            def load_x(i):
                X = xpool.tile([128, 4 * W], F16, tag="X", name=f"X_{i}")
                # two halves so compute can start on q0/q1 early
                nc.sync.dma_start(out=X[:, 0:2 * W], in_=x_in[i, :, 0:2 * W])
                nc.sync.dma_start(out=X[:, 2 * W:4 * W],
                                  in_=x_in[i, :, 2 * W:4 * W])
                return X

            Xg = {i: load_x(i) for i in range(min(PREF - 1, IMGS))}
            nc.gpsimd.dma_start(out=Wt[:], in_=w_in[:])
            Wr = Wt[:]

            # Tiny PE op consuming the weight DMA so later matmuls depend
            # on it via PE program order.
            warm = prpool.tile([4, 512], F32, tag="pR", bufs=1)
            nc.tensor.matmul(warm[0:1, 0:1], lhsT=Wr[:, 0:1], rhs=Wr[:, 0:1],
                             start=True, stop=True)

            def copy(dst, src, eng):
                if eng == "s":
                    nc.scalar.copy(out=dst, in_=src)
                else:
                    nc.vector.tensor_copy(out=dst, in_=src)

            def chain(lhsT_fn, M, np_, ts):
                """Banded filter-interleaved accumulation chain: cols
                [0,512) into main tile M (np_ partitions), tail [512,516)
                into tail slot ts.  lhsT_fn(q) gives the stationary for
                chunk q."""
                n = len(SEGS_MAIN)
                for si, (q, lo, hi) in enumerate(SEGS_MAIN):
                    nc.tensor.matmul(M[:, lo:hi], lhsT=lhsT_fn(q),
                                     rhs=_wslice(Wr, q, lo, hi),
                                     start=(si == 0), stop=(si == n - 1))
                q, lo, hi = SEG_TAIL
                nc.tensor.matmul(Tb[0:np_, 4 * ts:4 * ts + 4],
                                 lhsT=lhsT_fn(q), rhs=_wslice(Wr, q, lo, hi),
                                 start=True, stop=True)

            def pass1(i, Yt):
                Xv = Xg[i][:]
                for cc in range(4):
                    M = p1pool.tile([128, 512], F32, tag="p1")
                    chain(lambda q: Xv[:, q * W + cc * 128:q * W + (cc + 1) * 128],
                          M[:], 128, cc)
                    # contiguous [128,512] drain; split scalar/vector
                    copy(Yt[:, cc * 516:cc * 516 + 512], M[:],
                         "v" if cc != 3 else "s")
                # tail slots 0..3 -> Yt cols {cc*516+512..516}
                Ytv = Yt[:].rearrange("p (cc k) -> p cc k", cc=4)
                copy(Ytv[:, :, 512:516], Tb[:, 0:16].rearrange(
                    "p (cc k) -> p cc k", cc=4), "s")

            def pass2(i, Yt, STG):
                # col = cc*516 + 2*k + f  (k in [0,258), f interleaved)
                Ytv = Yt[:].rearrange("p (cc k f) -> p cc k f", cc=4, f=2)
                for blk in range(4):
                    khc, f = blk // 2, blk % 2
                    M = p2pool.tile([128, 512], F32, tag="p2")
                    chain(lambda q: Ytv[:, q, 128 * khc:128 * (khc + 1), f],
                          M[:], 128, 4 + blk)
                    copy(STG[:, blk * 516:blk * 516 + 512], M[:],
                         "v" if blk != 3 else "s")
                Sv = STG[:].rearrange("p (blk k) -> p blk k", blk=4)
                copy(Sv[:, :, 512:516], Tb[:, 16:32].rearrange(
                    "p (blk k) -> p blk k", blk=4), "s")

            def rem(i, Yt):
                # kh in {256,257}: lhsT = the 4 tail cols of each Yt block
                Ytv = Yt[:].rearrange("p (cc k) -> p cc k", cc=4)
                M = prpool.tile([4, 512], F32, tag="pR")
                chain(lambda q: Ytv[:, q, 512:516], M[:], 4, 8)
                base = i * 516
                copy(Crem[:, base:base + 512], M[:], "s")
                copy(Crem[:, base + 512:base + 516], Tb[0:4, 32:36], "v")

            # software pipeline: PE runs pass1(i+1) before pass2(i)
            Ytg = {0: ypool.tile([128, 4 * 516], F16, tag="Yt", name="Yt_0")}
            pass1(0, Ytg[0])
            for i in range(IMGS):
                if i + PREF - 1 < IMGS:
                    Xg[i + PREF - 1] = load_x(i + PREF - 1)
                if i + 1 < IMGS:
                    Ytg[i + 1] = ypool.tile([128, 4 * 516], F16, tag="Yt",
                                            name=f"Yt_{i + 1}")
                    pass1(i + 1, Ytg[i + 1])
                STG = spool.tile([128, 4 * 516], F16, tag="STG")
                pass2(i, Ytg[i], STG)
                rem(i, Ytg[i])
                del Ytg[i]
                ring = nc.sync if i == IMGS - 1 else nc.gpsimd
                ring.dma_start(out=y_out[i], in_=STG[:])
                if i % 8 == 7:
                    c = i // 8
                    nc.gpsimd.dma_start(
                        out=yr_out[:, c * 8 * 516:(c + 1) * 8 * 516],
                        in_=Crem[:, c * 8 * 516:(c + 1) * 8 * 516])
    nc.finalize()
    return nc


def _get_module() -> bass.Bass:
    global _MODULE
    if _MODULE is None:
        _MODULE = _build_module()
    return _MODULE


def make_in_maps(x: np.ndarray) -> list[dict]:
    x = np.asarray(x, dtype=np.float32)
    B, C, Hx, Wx = x.shape
    assert (Hx, Wx) == (H, W) and B * C == N_CORES * IMGS
    imgs = x.reshape(B * C, H, W)
    maps = []
    for k in range(N_CORES):
        # X[i][p, q*512 + c] = x[i, 128q + p, c]
        xc = imgs[k * IMGS:(k + 1) * IMGS].reshape(IMGS, 4, 128, W)
        xc = np.ascontiguousarray(xc.transpose(0, 2, 1, 3))
        maps.append({"x": xc.reshape(IMGS, 128, 4 * W).astype(np.float16),
                     "w": _WEIGHTS})
    return maps


def kernel(**inputs) -> np.ndarray:
    x = np.asarray(inputs["x"], dtype=np.float32)
    B, C, Hx, Wx = x.shape

    nc = _get_module()
    in_maps = make_in_maps(x)
    res = run_bass_kernel_spmd(nc, in_maps, list(range(N_CORES))).results

    full = np.empty((N_CORES * IMGS, 4, OUT, OUT), dtype=np.float32)
    for k in range(N_CORES):
        # ym[i, p, khc, f, kw, g] = O_{f+2g}[128*khc+p, kw]
        ym = res[k]["y"].reshape(IMGS, 128, 2, 2, OUT, 2)
        # yr[2*j + f, i, kw, g] = O_{f+2g}[256+j, kw]
        yr = res[k]["yr"].reshape(2, 2, IMGS, OUT, 2)
        dst = full[k * IMGS:(k + 1) * IMGS]
        # dst[i, 2g+f, khc*128+p, kw]
        t = ym.transpose(0, 5, 3, 2, 1, 4).reshape(IMGS, 4, 256, OUT)
        dst[:, :, :256, :] = t
        # remainder rows: yr[j, f, i, kw, g] -> dst[i, 2g+f, 256+j, kw]
        r = yr.transpose(2, 4, 1, 0, 3)  # [i, g, f, j, kw]
        dst[:, :, 256:258, :] = r.reshape(IMGS, 4, 2, OUT)

    return np.ascontiguousarray(full.reshape(B, 4 * C, OUT, OUT))


# revision 7
# speedup vs baseline: 1.1775x; 1.1775x over previous
"""Trainium2 Bass kernel for 2D single-level DWT (coif1, symmetric padding).

Input  x: (4, 64, 512, 512) fp32
Output  : (4, 256, 258, 258) fp32  -- per input channel: [cA, cH, cV, cD]

Math: with R_f the banded 258x512 operator of the 1D DWT along an axis
(6-tap filter, stride 2, symmetric boundary folds), the four outputs are
    cA = R_lo X R_lo^T,  cH = R_hi X R_lo^T,
    cV = R_lo X R_hi^T,  cD = R_hi X R_hi^T.

v5 design (per-image pipeline, contiguous 1-bank PSUM chains):
  pass 1 (contract rows r):   Yt_f[c, kh] = sum_r X[r, c] R_f[kh, r]
     stationary lhsT = X chunk [r:128, c:128]; moving rhs = R^T slice with
     the lo/hi filter pair interleaved along the stream dim (col 2*kh+f).
     R is banded: r-chunk q only reaches khf cols [128q, 128q+132).
  pass 2 (contract cols c):   O[kh, kwg] = sum_c Yt_f[c, kh] R_g[kw, c]
     stationary lhsT = stride-2 slice of Yt (kh chunk for filter f);
     moving rhs = the SAME banded weight tensor.
  Each chain writes khf/kwg cols [0,512) into ONE psum bank and the
  4-col tail [512,516) into a slot of a shared tail bank, so the main
  drain is a single contiguous [128,512] fp32->fp16 copy (strided drains
  cost ~2x on ACT/DVE).  Tail slots are drained once per image with one
  small strided copy.  Drains are split across scalar+vector by time.
  The PE runs pass1(i+1) before pass2(i) so drains trail a full image
  behind the producer.  DMA: per-image transfers (input 2 halves on the
  sync queue, output on gpsimd), input prefetch depth 5.
  A post-legalize pass drops LDWEIGHTS whose weights AP equals the
  previous load on the final PE stream (the PE keeps its stationary
  across matmuls), saving ~105ns of queue occupancy each.
"""

import os
import sys

for _p in ("/opt/trn_rl_repo", "/opt/pypackages"):
    if _p not in sys.path:
        sys.path.append(_p)

os.environ.setdefault("JAX_COMPILATION_CACHE_DIR", "/tmp/jax_comp_cache")
os.environ.setdefault("JAX_PERSISTENT_CACHE_MIN_COMPILE_TIME_SECS", "10")

import numpy as np

import concourse.bass as bass
import concourse.bacc as bacc
import concourse.mybir as mybir
import concourse.tile as _tile_mod
from concourse.bass_utils import run_bass_kernel_spmd
from concourse.tile import TileContext

N_CORES = 8
H = W = 512
OUT = 258  # (512 + 6 - 1) // 2
IMGS = 32  # images per core (4*64/8)
F16 = mybir.dt.float16
F32 = mybir.dt.float32

# pywt coif1 decomposition filters
DEC_LO = np.array([-0.01565572813546454, -0.0727326195128539, 0.38486484686420286,
                   0.8525720202122554, 0.3378976624578092, -0.0727326195128539])
DEC_HI = np.array([0.0727326195128539, 0.3378976624578092, -0.8525720202122554,
                   0.38486484686420286, 0.0727326195128539, -0.01565572813546454])
FLEN = 6
PAD = 4
LO_F = DEC_LO[::-1]
HI_F = DEC_HI[::-1]

# kh/kw window that r/c-chunk q contributes to (from the band structure)
WINS = [(0, 66), (64, 130), (128, 194), (192, 258)]


def _dedup_ldweights(ordered, nc):
    """Drop InstLdweights whose weights AP is identical to the previous
    PE weight load in the final post-schedule order (only matmuls in
    between).  The PE array keeps its stationary operand across matmuls,
    so the reload is a no-op; removal is done only when the candidate's
    dep edges are covered by the live load and nothing depends on it."""
    PE = mybir.EngineType.PE
    for bb, insts in ordered.items():
        cur_sig = None
        cur_deps = None
        keep = []
        for inst in insts:
            if getattr(inst, "engine", None) != PE:
                keep.append(inst)
                continue
            tn = type(inst).__name__
            if tn == "InstLdweights":
                sig = str(inst.ins[0])
                deps = (frozenset(inst.sync_dependency_names()),
                        frozenset(inst.nosync_dependency_names()))
                if (cur_sig is not None and sig == cur_sig
                        and deps[0] <= cur_deps[0] and deps[1] <= cur_deps[1]
                        and not inst.descendants):
                    continue
                cur_sig, cur_deps = sig, deps
                keep.append(inst)
            elif tn == "InstMatmult":
                if inst.is_transpose:
                    cur_sig = None
                keep.append(inst)
            else:
                keep.append(inst)
        ordered[bb] = keep
    return ordered


_orig_tile_legalize = _tile_mod.tile_legalize


def _legalize_with_dedup(ordered, nc):
    return _dedup_ldweights(_orig_tile_legalize(ordered, nc), nc)


_tile_mod.tile_legalize = _legalize_with_dedup


def _build_R(filt: np.ndarray, n: int = W) -> np.ndarray:
    """Banded [258, 512] operator: out[k] = sum_j filt[j] * x[sym(2k + j - PAD)]."""
    out_len = (n + FLEN - 1) // 2

    def sym(i: int) -> int:
        while i < 0 or i >= n:
            if i < 0:
                i = -i - 1
            if i >= n:
                i = 2 * n - 1 - i
        return i

    R = np.zeros((out_len, n), dtype=np.float64)
    for k in range(out_len):
        for j in range(FLEN):
            R[k, sym(2 * k + j - PAD)] += filt[j]
    return R


def _build_weights() -> np.ndarray:
    """Interleaved: w[p, q*516 + 2k + f] = R_f[k, 128q + p], [128, 4*516] fp16."""
    Rs = [_build_R(LO_F), _build_R(HI_F)]
    w = np.zeros((128, 4 * 2 * OUT), dtype=np.float32)
    for q in range(4):
        blk = np.zeros((128, OUT, 2), dtype=np.float32)
        for f in range(2):
            blk[:, :, f] = Rs[f][:, 128 * q:128 * (q + 1)].T
        w[:, q * 2 * OUT:(q + 1) * 2 * OUT] = blk.reshape(128, 2 * OUT)
    return w.astype(np.float16)


_WEIGHTS = _build_weights()
_MODULE = None

# chain segments: (q, psum col lo, psum col hi) over the khf/kwg index
# [0,516); q's band is [128q, 128q+132), clipped at the 512 bank edge.
# The [512,516) remainder goes to a tail-bank slot.
SEGS_MAIN = [(0, 0, 132), (1, 128, 260), (2, 256, 388), (3, 384, 512)]
SEG_TAIL = (3, 512, 516)


def _wslice(Wr, q, lo, hi):
    base = q * 516 + 128 * q
    return Wr[:, base + (lo - 128 * q):base + (hi - 128 * q)]


def _build_module() -> bass.Bass:
    nc = bacc.Bacc("TRN2", target_bir_lowering=False, debug=False)
    x_in = nc.declare_dram_parameter("x", [IMGS, 128, 4 * W], F16, isOutput=False)
    w_in = nc.declare_dram_parameter("w", [128, 4 * 516], F16, isOutput=False)
    # y[i, p, (2*khc+f)*516 + 2*kw + g] = O_{f+2g}[128*khc + p, kw]
    y_out = nc.declare_dram_parameter("y", [IMGS, 128, 4 * 516], F16,
                                      isOutput=True)
    # yr[2*j + f, i*516 + 2*kw + g] = O_{f+2g}[256 + j, kw]
    yr_out = nc.declare_dram_parameter("yr", [4, IMGS * 516], F16,
                                       isOutput=True)

    PREF = 5  # input prefetch depth (images)

    with TileContext(nc) as tc:
        with (
            tc.tile_pool(name="wpool", bufs=1) as wpool,
            tc.tile_pool(name="xpool", bufs=PREF) as xpool,
            tc.tile_pool(name="ypool", bufs=3) as ypool,
            tc.tile_pool(name="spool", bufs=3) as spool,
            tc.tile_pool(name="rpool", bufs=1) as rpool,
            tc.tile_pool(name="p1", bufs=2, space="PSUM") as p1pool,
            tc.tile_pool(name="p2", bufs=2, space="PSUM") as p2pool,
        ):
            Wt = wpool.tile([128, 4 * 516], F16)
            Crem = rpool.tile([4, IMGS * 516], F16)

            def load_x(i):
                X = xpool.tile([128, 4 * W], F16, tag="X", name=f"X_{i}")
                # two halves so compute can start on q0/q1 early
                nc.sync.dma_start(out=X[:, 0:2 * W], in_=x_in[i, :, 0:2 * W])
                nc.sync.dma_start(out=X[:, 2 * W:4 * W],
                                  in_=x_in[i, :, 2 * W:4 * W])
                return X

            Xg = {i: load_x(i) for i in range(min(PREF - 1, IMGS))}
            nc.gpsimd.dma_start(out=Wt[:], in_=w_in[:])
            Wr = Wt[:]

            # Tiny PE op consuming the weight DMA so later matmuls depend
            # on it via PE program order.
            warm = p1pool.tile([128, 1024], F32, tag="p1")
            nc.tensor.matmul(warm[0:1, 0:1], lhsT=Wr[:, 0:1], rhs=Wr[:, 0:1],
                             start=True, stop=True)

            def copy(dst, src, eng):
                if eng == "s":
                    nc.scalar.copy(out=dst, in_=src)
                else:
                    nc.vector.tensor_copy(out=dst, in_=src)

            def chain(lhsT_fn, M):
                """Banded filter-interleaved accumulation chain into cols
                [0,516) of the 2-bank tile M: [0,512) accumulates in bank
                0, the [512,516) tail is a single fresh-write matmul into
                bank 1, so the drain is one contiguous [*,516] copy."""
                n = len(SEGS_MAIN)
                for si, (q, lo, hi) in enumerate(SEGS_MAIN):
                    nc.tensor.matmul(M[:, lo:hi], lhsT=lhsT_fn(q),
                                     rhs=_wslice(Wr, q, lo, hi),
                                     start=(si == 0), stop=(si == n - 1))
                q, lo, hi = SEG_TAIL
                nc.tensor.matmul(M[:, lo:hi], lhsT=lhsT_fn(q),
                                 rhs=_wslice(Wr, q, lo, hi),
                                 start=True, stop=True)

            def pass1(i, Yt):
                Xv = Xg[i][:]
                for cc in range(4):
                    M = p1pool.tile([128, 1024], F32, tag="p1")
                    chain(lambda q: Xv[:, q * W + cc * 128:q * W + (cc + 1) * 128],
                          M[:])
                    # one contiguous [128,516] drain; alternate engines
                    copy(Yt[:, cc * 516:(cc + 1) * 516], M[:, 0:516],
                         "v" if cc % 2 == 0 else "s")

            def pass2(i, Yt, STG):
                # col = cc*516 + 2*k + f  (k in [0,258), f interleaved)
                Ytv = Yt[:].rearrange("p (cc k f) -> p cc k f", cc=4, f=2)
                for blk in range(4):
                    khc, f = blk // 2, blk % 2
                    M = p2pool.tile([128, 1024], F32, tag="p2")
                    chain(lambda q: Ytv[:, q, 128 * khc:128 * (khc + 1), f],
                          M[:])
                    copy(STG[:, blk * 516:(blk + 1) * 516], M[:, 0:516],
                         "v" if blk % 2 == 0 else "s")

            def rem(i, Yt):
                # kh in {256,257}: lhsT = the 4 tail cols of each Yt block
                Ytv = Yt[:].rearrange("p (cc k) -> p cc k", cc=4)
                M = p1pool.tile([128, 1024], F32, tag="p1")
                chain(lambda q: Ytv[:, q, 512:516], M[0:4, :])
                copy(Crem[:, i * 516:(i + 1) * 516], M[0:4, 0:516], "s")

            # software pipeline: PE runs pass1(i+1) before pass2(i)
            Ytg = {0: ypool.tile([128, 4 * 516], F16, tag="Yt", name="Yt_0")}
            pass1(0, Ytg[0])
            for i in range(IMGS):
                if i + PREF - 1 < IMGS:
                    Xg[i + PREF - 1] = load_x(i + PREF - 1)
                if i + 1 < IMGS:
                    Ytg[i + 1] = ypool.tile([128, 4 * 516], F16, tag="Yt",
                                            name=f"Yt_{i + 1}")
                    pass1(i + 1, Ytg[i + 1])
                STG = spool.tile([128, 4 * 516], F16, tag="STG")
                pass2(i, Ytg[i], STG)
                rem(i, Ytg[i])
                del Ytg[i]
                ring = nc.sync if i == IMGS - 1 else nc.gpsimd
                ring.dma_start(out=y_out[i], in_=STG[:])
                if i % 8 == 7:
                    c = i // 8
                    nc.gpsimd.dma_start(
                        out=yr_out[:, c * 8 * 516:(c + 1) * 8 * 516],
                        in_=Crem[:, c * 8 * 516:(c + 1) * 8 * 516])
    nc.finalize()
    return nc


def _get_module() -> bass.Bass:
    global _MODULE
    if _MODULE is None:
        _MODULE = _build_module()
    return _MODULE


def make_in_maps(x: np.ndarray) -> list[dict]:
    x = np.asarray(x, dtype=np.float32)
    B, C, Hx, Wx = x.shape
    assert (Hx, Wx) == (H, W) and B * C == N_CORES * IMGS
    imgs = x.reshape(B * C, H, W)
    maps = []
    for k in range(N_CORES):
        # X[i][p, q*512 + c] = x[i, 128q + p, c]
        xc = imgs[k * IMGS:(k + 1) * IMGS].reshape(IMGS, 4, 128, W)
        xc = np.ascontiguousarray(xc.transpose(0, 2, 1, 3))
        maps.append({"x": xc.reshape(IMGS, 128, 4 * W).astype(np.float16),
                     "w": _WEIGHTS})
    return maps


def kernel(**inputs) -> np.ndarray:
    x = np.asarray(inputs["x"], dtype=np.float32)
    B, C, Hx, Wx = x.shape

    nc = _get_module()
    in_maps = make_in_maps(x)
    res = run_bass_kernel_spmd(nc, in_maps, list(range(N_CORES))).results

    full = np.empty((N_CORES * IMGS, 4, OUT, OUT), dtype=np.float32)
    for k in range(N_CORES):
        # ym[i, p, khc, f, kw, g] = O_{f+2g}[128*khc+p, kw]
        ym = res[k]["y"].reshape(IMGS, 128, 2, 2, OUT, 2)
        # yr[2*j + f, i, kw, g] = O_{f+2g}[256+j, kw]
        yr = res[k]["yr"].reshape(2, 2, IMGS, OUT, 2)
        dst = full[k * IMGS:(k + 1) * IMGS]
        # dst[i, 2g+f, khc*128+p, kw]
        t = ym.transpose(0, 5, 3, 2, 1, 4).reshape(IMGS, 4, 256, OUT)
        dst[:, :, :256, :] = t
        # remainder rows: yr[j, f, i, kw, g] -> dst[i, 2g+f, 256+j, kw]
        r = yr.transpose(2, 4, 1, 0, 3)  # [i, g, f, j, kw]
        dst[:, :, 256:258, :] = r.reshape(IMGS, 4, 2, OUT)

    return np.ascontiguousarray(full.reshape(B, 4 * C, OUT, OUT))
